# revision 1
# baseline (speedup 1.0000x reference)
"""Trainium2 Bass kernel for nn_Downsample_PASA_group_softmax (pooling).

Full-input contract: kernel(**inputs) takes the complete batch (n=8) and
returns the full output. Sharding: pure data parallelism, one sample per
NeuronCore across 8 cores (same Bass/Tile program, per-core in_maps).

Per-core pipeline (channels on partitions, pixels on free dim, fp16 on chip):
  conv3x3 -> 18 PSUM-accumulated PE matmuls; BN+exp fused on ScalarE;
  softmax denominator via all-ones PE matmul (x1/256 to stay in fp16 range)
  + reciprocal_approx_fast; per-(group,tap) sigma row broadcast 18->128
  partitions via PE selector matmuls; ScalarE evacuates PSUM->SBUF; VectorE
  does the 9 multiply + 8 add passes against reflect-padded, column-shifted
  fp16 copies of x. Emission is software-pipelined: pooling of super-block
  sb-1 interleaves with conv of sb so ScalarE's in-order queue never stalls
  the conv chain. Measured ~510 us/core steady-state on HW.
"""

import numpy as np
from contextlib import ExitStack

N_CORES = 8

"""Bass/Tile kernel builder for PASA group-softmax downsample (shared by dev + kernel.py).

Per-core problem (one sample):
  x (256, 128, 128) f32  -> out (256, 16384) f32
  logits = conv3x3(reflect_pad(x), W)            # (18, 16384)
  E = exp(bn_scale * logits + bn_bias)           # fp16
  sigma = E / sum_k E                            # softmax over all 18
  out[c, q] = sum_k sigma[g(c)*9+k, q] * xpad[c, q + off_k]

On-chip layout: channels on partitions (2 group tiles of 128), pixels on free dim.
xp = row-reflect-padded fp16 image [128, 130 rows x 128 cols] (pitch 128 so the
interior loads as ONE contiguous DMA). Column shifts (kw=0/2) come from per-16-row
left/right shifted copies (SBUF->SBUF DMA, +-1 element) with a tiny strided DVE
copy fixing the reflect edge column. sigma is broadcast 18 -> 128 partitions via
PE selector matmuls; ScalarE evacuates PSUM->SBUF; VectorE does fp16 mult/add.
"""

import numpy as np
from contextlib import ExitStack

import concourse.mybir as mybir

FP16 = mybir.dt.float16
FP32 = mybir.dt.float32
AF = mybir.ActivationFunctionType
ALU = mybir.AluOpType

C = 256
H = W = 128
Q = H * W              # 16384 pixels
G = 2
K = 3
NK = K * K             # 9
NO = G * NK            # 18 conv outputs
PROWS = 130            # padded rows (row -1 and 128 reflect)
BASE = 2               # front pad elems (keeps 4B alignment, allows -1 shift)
XPLEN = BASE + PROWS * W + 2

SB_ROWS = 32           # super-block = 32 output rows
N_SB = H // SB_ROWS    # 8
SPAN = SB_ROWS * W     # 2048 px
CHUNK = 512            # conv / softmax-prep chunk (4 rows)
HALF = 1024            # sigma-bcast evac chunk
VROWS = SB_ROWS + 2    # shifted-variant rows per super-block


def host_constants(conv_w, gamma, beta, run_mean, run_var):
    w = np.asarray(conv_w, np.float32)
    lhsT_conv = np.zeros((128, NO * NO), np.float16)
    for g in range(G):
        for kh in range(K):
            for kw in range(K):
                mm = g * NK + kh * K + kw
                blk = w[:, g * 128:(g + 1) * 128, kh, kw].T  # (128, 18)
                lhsT_conv[:, mm * NO:(mm + 1) * NO] = blk.astype(np.float16)
    sel = np.zeros((NO, NO * 128), np.float16)
    for s in range(NO):
        sel[s, s * 128:(s + 1) * 128] = 1.0
    ones18 = np.full((NO, NO), 1.0 / 256.0, np.float16)
    scale = np.asarray(gamma, np.float32) / np.sqrt(np.asarray(run_var, np.float32) + 1e-5)
    bias = np.asarray(beta, np.float32) - np.asarray(run_mean, np.float32) * scale
    return {
        "lhsT_conv": lhsT_conv,
        "sel": sel,
        "ones18": ones18,
        "bn_scale": scale.reshape(NO, 1).astype(np.float32),
        "bn_bias": bias.reshape(NO, 1).astype(np.float32),
    }


def declare_io(nc):
    ins = {
        "x": nc.dram_tensor("x", (C, H, W), FP32, kind="ExternalInput").ap(),
        "lhsT_conv": nc.dram_tensor("lhsT_conv", (128, NO * NO), FP16, kind="ExternalInput").ap(),
        "sel": nc.dram_tensor("sel", (NO, NO * 128), FP16, kind="ExternalInput").ap(),
        "ones18": nc.dram_tensor("ones18", (NO, NO), FP16, kind="ExternalInput").ap(),
        "bn_scale": nc.dram_tensor("bn_scale", (NO, 1), FP32, kind="ExternalInput").ap(),
        "bn_bias": nc.dram_tensor("bn_bias", (NO, 1), FP32, kind="ExternalInput").ap(),
    }
    out = nc.dram_tensor("out", (C, Q), FP32, kind="ExternalOutput").ap()
    return ins, out


def make_pools(ctx: ExitStack, tc):
    p = {}
    p["const"] = ctx.enter_context(tc.tile_pool(name="const", bufs=1))
    p["xp"] = ctx.enter_context(tc.tile_pool(name="xp", bufs=1))
    p["e"] = ctx.enter_context(tc.tile_pool(name="e", bufs=2))
    p["xvar"] = ctx.enter_context(tc.tile_pool(name="xvar", bufs=2))
    p["rchunk"] = ctx.enter_context(tc.tile_pool(name="rchunk", bufs=2))
    p["sgb"] = ctx.enter_context(tc.tile_pool(name="sgb", bufs=3))
    p["acc"] = ctx.enter_context(tc.tile_pool(name="acc", bufs=3))
    p["psc"] = ctx.enter_context(tc.tile_pool(name="psc", bufs=2, space="PSUM"))
    p["psb"] = ctx.enter_context(tc.tile_pool(name="psb", bufs=3, space="PSUM"))
    return p


def load_consts(tc, p, in_aps):
    nc = tc.nc
    const = p["const"]
    c = {}
    c["lhsT_conv"] = const.tile([128, NO * NO], FP16, tag="lhsT_conv", name="lhsT_conv")
    nc.sync.dma_start(c["lhsT_conv"][:], in_aps["lhsT_conv"][:])
    c["sel"] = const.tile([NO, NO * 128], FP16, tag="sel", name="sel")
    nc.sync.dma_start(c["sel"][:], in_aps["sel"][:])
    c["ones18"] = const.tile([NO, NO], FP16, tag="ones18", name="ones18")
    nc.sync.dma_start(c["ones18"][:], in_aps["ones18"][:])
    c["bn_scale"] = const.tile([NO, 1], FP32, tag="bn_scale", name="bn_scale")
    nc.sync.dma_start(c["bn_scale"][:], in_aps["bn_scale"][:])
    c["bn_bias"] = const.tile([NO, 1], FP32, tag="bn_bias", name="bn_bias")
    nc.sync.dma_start(c["bn_bias"][:], in_aps["bn_bias"][:])
    return c


ABL_skip_dve_pool = False
ABL_skip_evac = False
ABL_skip_bcast = False
ABL_skip_conv = False


def emit_body(tc, p, c, out_ap, in_aps):
    nc = tc.nc
    x_d = in_aps["x"]
    e_pool, var_pool, r_pool = p["e"], p["xvar"], p["rchunk"]
    sgb_pool, acc_pool = p["sgb"], p["acc"]
    xp_pool, ps_conv, ps_b = p["xp"], p["psc"], p["psb"]
    lhsT_conv, sel, ones18 = c["lhsT_conv"], c["sel"], c["ones18"]
    bn_scale, bn_bias = c["bn_scale"], c["bn_bias"]

    # ---- row-reflect-padded input (fp16, pitch 128), per group ----
    xp = []
    xv = []  # 3D views [128, 130, 128]
    for g in range(G):
        t = xp_pool.tile([128, XPLEN], FP16, tag=f"xp{g}")
        xp.append(t)
        nc.vector.memset(t[:, 0:BASE], 0.0)
        nc.vector.memset(t[:, XPLEN - 2:XPLEN], 0.0)
        v = t[:, BASE:BASE + PROWS * W].rearrange("p (r j) -> p r j", j=W)
        xv.append(v)
        # interior (rows 1..128) = x, one contiguous DMA with fp32->fp16 cast
        nc.gpsimd.dma_start(v[:, 1:129, :], x_d[g * 128:(g + 1) * 128, :, :])
        # row reflect: pad row -1 <- x row 1 (view row 2), pad row 128 <- x row 126
        nc.vector.tensor_copy(v[:, 0:1, :], v[:, 2:3, :])
        nc.vector.tensor_copy(v[:, 129:130, :], v[:, 127:128, :])

    def emit_conv_chunk(sb, cc, E, rhs_ap, st=None):
        r0 = sb * SB_ROWS + cc * 4
        eq0 = cc * 4 * W
        cps = ps_conv.tile([32 + NO, CHUNK], FP32, tag="conv", name="cps")
        idx = 0
        for g in range(G):
            for kh in range(K):
                for kw in range(K):
                    mm = g * NK + kh * K + kw
                    nc.tensor.matmul(
                        cps[0:NO, :],
                        lhsT_conv[:, mm * NO:(mm + 1) * NO],
                        rhs_ap(g, kh, kw, r0, 4),
                        start=(idx == 0),
                        stop=(idx == NO - 1),
                    )
                    idx += 1
        nc.scalar.activation(E[:, eq0:eq0 + CHUNK], cps[0:NO, :], AF.Exp,
                             bias=bn_bias[:], scale=bn_scale[:])
        nc.tensor.matmul(cps[32:32 + NO, :], ones18[:], E[:, eq0:eq0 + CHUNK],
                         start=True, stop=True)
        rch = r_pool.tile([NO, CHUNK], FP16, tag="r", name="rch")
        with nc.allow_low_precision("softmax recip in fp16"):
            nc.vector.reciprocal(rch[:], cps[32:32 + NO, :])
        nc.vector.scalar_tensor_tensor(
            E[:, eq0:eq0 + CHUNK], E[:, eq0:eq0 + CHUNK], 1.0 / 256.0, rch[:],
            ALU.mult, ALU.mult,
        )

    def emit_pool_unit(st, u):
        """u in 0..17 -> (g, k); st = dict(sb=, E=, rhs_ap=, accs={g: tile})"""
        sb, E, rhs_ap = st["sb"], st["E"], st["rhs_ap"]
        g, k = divmod(u, NK)
        kh, kw = divmod(k, K)
        s = g * NK + k
        p0 = sb * SPAN
        if k == 0:
            st["accs"][g] = acc_pool.tile([128, SPAN], FP16, tag="acc", name="acc")
        acc = st["accs"][g]
        sgb = sgb_pool.tile([128, SPAN], FP16, tag="sgb", name="sgb")
        xin = rhs_ap(g, kh, kw, sb * SB_ROWS, SB_ROWS)
        if not ABL_skip_bcast:
            for h in range(SPAN // HALF):
                bps = ps_b.tile([128, HALF], FP32, tag="b", name="bps")
                for j in range(HALF // CHUNK):
                    qq = h * HALF + j * CHUNK
                    nc.tensor.matmul(bps[:, j * CHUNK:(j + 1) * CHUNK],
                                     sel[:, s * 128:(s + 1) * 128],
                                     E[:, qq:qq + CHUNK], start=True, stop=True)
                if not ABL_skip_evac:
                    nc.scalar.copy(sgb[:, h * HALF:(h + 1) * HALF], bps[:])
        if not ABL_skip_dve_pool:
            if k == 0:
                nc.vector.tensor_mul(acc[:], sgb[:], xin)
            else:
                nc.vector.tensor_mul(sgb[:], sgb[:], xin)
                nc.vector.tensor_add(acc[:], acc[:], sgb[:])
        elif k == 0:
            nc.vector.memset(acc[:, 0:2], 0.0)
        if k == NK - 1:
            nc.gpsimd.dma_start(out_ap[g * 128:(g + 1) * 128, p0:p0 + SPAN], acc[:])

    def make_sb_state(sb):
        E = e_pool.tile([NO, SPAN], FP16, tag="e", name="E")
        vflat = {}
        r_off = sb * SB_ROWS * W
        for g in range(G):
            xl = var_pool.tile([128, VROWS * W], FP16, tag=f"xl{g}", name="xl")
            nc.sync.dma_start(xl[:], xp[g][:, BASE - 1 + r_off:BASE - 1 + r_off + VROWS * W])
            xlv = xl[:].rearrange("p (r j) -> p r j", j=W)
            nc.vector.tensor_copy(xlv[:, :, 0:1], xv[g][:, sb * SB_ROWS:sb * SB_ROWS + VROWS, 1:2])
            xr = var_pool.tile([128, VROWS * W], FP16, tag=f"xr{g}", name="xr")
            nc.sync.dma_start(xr[:], xp[g][:, BASE + 1 + r_off:BASE + 1 + r_off + VROWS * W])
            xrv = xr[:].rearrange("p (r j) -> p r j", j=W)
            nc.vector.tensor_copy(xrv[:, :, 127:128], xv[g][:, sb * SB_ROWS:sb * SB_ROWS + VROWS, 126:127])
            vflat[(g, 0)] = xl
            vflat[(g, 2)] = xr

        def rhs_ap(g, kh, kw, r0, nrows, _sb=sb, _vf=vflat):
            if kw == 1:
                o = BASE + (r0 + kh) * W
                return xp[g][:, o:o + nrows * W]
            vf = _vf[(g, kw)]
            rl = r0 - _sb * SB_ROWS
            return vf[:, (rl + kh) * W:(rl + kh + nrows) * W]

        return {"sb": sb, "E": E, "rhs_ap": rhs_ap, "accs": {}}

    # software-pipelined emission: pooling of sb-1 interleaved into conv of sb
    UNIT_SCHED = [3, 3, 2, 2, 2, 2, 2, 2]  # pool units after each conv chunk
    prev = None
    for sb in range(N_SB):
        st = make_sb_state(sb)
        conv_iter = range(0) if ABL_skip_conv else range(SB_ROWS // 4)
        u0 = 0
        for cc in conv_iter:
            emit_conv_chunk(sb, cc, st["E"], st["rhs_ap"], st)
            if prev is not None:
                for u in range(u0, u0 + UNIT_SCHED[cc]):
                    emit_pool_unit(prev, u)
                u0 += UNIT_SCHED[cc]
        if prev is not None:
            for u in range(u0, NO):
                emit_pool_unit(prev, u)
        prev = st
    for u in range(NO):
        emit_pool_unit(prev, u)


def build(ctx: ExitStack, tc, out_ap, in_aps):
    p = make_pools(ctx, tc)
    c = load_consts(tc, p, in_aps)
    emit_body(tc, p, c, out_ap, in_aps)


_COMPILED = {}


def _get_compiled():
    if "nc" not in _COMPILED:
        import concourse.bacc as bacc
        import concourse.tile as tile

        nc = bacc.Bacc("TRN2", target_bir_lowering=False, debug=False,
                       num_devices=N_CORES)
        ins, out_ap = declare_io(nc)
        with tile.TileContext(nc) as tc:
            with ExitStack() as ctx:
                build(ctx, tc, out_ap, ins)
        nc.compile()
        _COMPILED["nc"] = nc
    return _COMPILED["nc"]


def kernel(x, conv_w, gamma, beta, run_mean, run_var):
    from concourse import bass_utils

    x = np.asarray(x, np.float32)
    n = x.shape[0]
    assert n == N_CORES, f"expected batch {N_CORES}, got {n}"
    consts = host_constants(np.asarray(conv_w, np.float32), np.asarray(gamma, np.float32),
                            np.asarray(beta, np.float32), np.asarray(run_mean, np.float32),
                            np.asarray(run_var, np.float32))
    nc = _get_compiled()
    in_maps = [{"x": x[i], **consts} for i in range(N_CORES)]
    res = bass_utils.run_bass_kernel_spmd(nc, in_maps, core_ids=list(range(N_CORES)))
    out = np.stack([res.results[i]["out"].reshape(C, H, W) for i in range(N_CORES)])
    return out.astype(np.float32)

